# revision 2
# baseline (speedup 1.0000x reference)
"""Trainium2 Bass kernel for nn_AET_34737695490187 (histogram_binning).

Math (per sample):
  d = bbox // 72; label = y0*14+x0 where (x0==x1 & y0==y1 & mask) else invalid
  segment-sum text_embeds into 196 patch buckets -> sums, counts
  tpe = sums / max(counts, 1)
  logits1 = ipe @ tpe^T  (per sample);  logits2 = logits1^T
  loss = mean of CE(logits, diag) both ways / 2

Sharding: pure data parallel, 8 samples per core on 8 NeuronCores.

Device pipeline ("G-path"), per core:
  - batched seg chain on DVE for all samples: d = (bbox*1821)>>17, validity,
    scatter index (int16) = label + 196*j + 784*(sample%2)
  - one-hot S [tokens, 196] built by gpsimd local_scatter, one per sample
    pair (tokens packed 4-per-partition)
  - per sample k:
      G = E @ ipe^T via TensorE (contraction over C), output [tok, 196]
          in the same 4-tokens-per-partition layout; copied to SBUF bf16
          with a constant ones column appended -> G1 [128, 4, 197+pad]
      LT_x[q, 0:197] = S^T @ [G | 1]: logits2 unscaled in cols 0:196 and
          the bucket counts in col 196 (one fused matmul chain)
      inv = 1/max(cnt,1) from LT_x col 196; LT scaled to bf16 via ACT
      CE both orientations: DVE -max, ACT exp(bias=-max, accum_out=s);
          transposed orientation via 4 TensorE transposes; diagonal via
          DVE scalar_tensor_tensor accumulate against an identity mask
  - final: Ln(s)-sums, column reduce, partition reduce via matmul with
    ones -> per-core scalar partial; host sums partials and normalizes.
"""

import numpy as np

B, L, C, P = 64, 512, 768, 196
NCORES = 8
SPC = B // NCORES          # samples per core
TPD = 4                    # tokens per partition (512 = 128 * 4)
NQ2 = P - 128              # 68
GW = 200                   # padded width of one G chunk (196 data + 1 ones + pad)
PX = P + 1                 # 197: logits + counts column

_NC_CACHE = None


def _build_nc():
    global _NC_CACHE
    if _NC_CACHE is not None:
        return _NC_CACHE

    import concourse.bass as bass
    import concourse.mybir as mybir
    from concourse import bacc, tile

    f32 = mybir.dt.float32
    bf16 = mybir.dt.bfloat16
    i32 = mybir.dt.int32
    i16 = mybir.dt.int16
    AF = mybir.ActivationFunctionType
    OP = mybir.AluOpType
    AX = mybir.AxisListType
    PSUM = bass.MemorySpace.PSUM

    nc = bacc.Bacc(target_bir_lowering=False, debug=False)

    # per-sample stream: E^T c-major [ci, j, m] (3072) then ipe^T (6*196)
    SW = 6 * TPD * 128 + 6 * P
    es_in = nc.dram_tensor("es", [128, SPC * SW], bf16, kind="ExternalInput")
    bbox_in = nc.dram_tensor("bboxq", [128, SPC * 16], i32, kind="ExternalInput")
    mask_in = nc.dram_tensor("maskq", [128, SPC * TPD], f32, kind="ExternalInput")
    imask_in = nc.dram_tensor("imask", [128, 392], bf16, kind="ExternalInput")
    off_in = nc.dram_tensor("offc", [128, 33], f32, kind="ExternalInput")
    out_ext = nc.dram_tensor("out", [1, 1], f32, kind="ExternalOutput")

    with tile.TileContext(nc) as tc:
        with (
            tc.tile_pool(name="const", bufs=1) as cpool,
            tc.tile_pool(name="e", bufs=6) as epool,
            tc.tile_pool(name="s4", bufs=4) as spool,
            tc.tile_pool(name="g1", bufs=4) as gpool,
            tc.tile_pool(name="ltb", bufs=8) as ltpool,
            tc.tile_pool(name="scr", bufs=8) as scrpool,
            tc.tile_pool(name="cols", bufs=6) as colpool,
            tc.tile_pool(name="acc", bufs=1) as accpool,
            tc.tile_pool(name="ps_g", bufs=3, space=PSUM) as ps_g,
            tc.tile_pool(name="ps_lt", bufs=3, space=PSUM) as ps_lt,
            tc.tile_pool(name="ps_tr", bufs=2, space=PSUM) as ps_tr,
        ):
            # ---- first sample's stream leads the sync DMA ring ----
            es0 = epool.tile([128, SW], bf16, tag="ett")
            nc.sync.dma_start(es0[:], es_in[:, 0:SW])

            # ---- small constants (DMA FIFO order matters) ----
            imaskc = cpool.tile([128, 392], bf16, tag="imask")
            nc.scalar.dma_start(imaskc[:], imask_in[:])
            offc = cpool.tile([128, 33], f32, tag="offc")
            nc.scalar.dma_start(offc[:], off_in[:])
            bboxa = cpool.tile([128, SPC * 16], i32, tag="bboxa")
            nc.sync.dma_start(bboxa[:], bbox_in[:])
            maska = cpool.tile([128, SPC * TPD], f32, tag="maska")
            nc.sync.dma_start(maska[:], mask_in[:])

            # warm the Exp + Ln activation tables off the critical path
            wdum = colpool.tile([1, 1], f32, tag="wdum")
            nc.scalar.activation(wdum[:], offc[0:1, 0:1], AF.Exp)
            wdum2 = colpool.tile([1, 1], f32, tag="wdum2")
            nc.scalar.activation(wdum2[:], offc[0:1, 0:1], AF.Ln)

            # warm the PE HAM clock gate: dummy matmuls on a zeroed tile
            # keep TensorE busy from t~0 until the first real matmuls arrive
            # (~79ns each; sized to end when sample 0's stream lands)
            NWARM = 58
            wsrc = cpool.tile([128, 128], bf16, tag="wsrc")
            nc.gpsimd.memset(wsrc[:], 0.0)
            wps = ps_tr.tile([128, 128], f32, tag="tr")
            for wi in range(NWARM):
                nc.tensor.matmul(wps[:], wsrc[:], wsrc[:],
                                 start=(wi == 0), stop=(wi == NWARM - 1))

            ident128 = imaskc[0:128, 0:128]
            ident68 = imaskc[0:68, 0:68]
            ones_f32 = offc[:, 32:33]

            # ---- persistent accumulators (unified; dead rows pre-set) ----
            scatA = accpool.tile([128, 4 * SPC], f32, tag="scatA")
            nc.gpsimd.memset(scatA[:], 1.0)
            mcatA = accpool.tile([128, 4 * SPC], f32, tag="mcatA")
            nc.gpsimd.memset(mcatA[:], 0.0)
            dcatA = accpool.tile([128, SPC], f32, tag="dcatA")

            # ---- seg chain for ALL samples at once (DVE) ----
            NT = SPC * 16
            mtile = colpool.tile([128, NT], i32, tag="mtile")
            nc.vector.tensor_scalar(mtile[:], bboxa[:], 1821, None, OP.mult)
            dtile = colpool.tile([128, NT], i32, tag="dtile")
            nc.vector.tensor_scalar(dtile[:], mtile[:], 17, None, OP.arith_shift_right)
            d3 = dtile[:].rearrange("p (t c) -> p t c", c=4)

            eqs = colpool.tile([128, NT // 2], f32, tag="eqs")
            eqs3 = eqs[:].rearrange("p (t c) -> p t c", c=2)
            nc.vector.tensor_tensor(eqs3, d3[:, :, 0:2], d3[:, :, 2:4], OP.is_equal)

            v1 = colpool.tile([128, NT // 4], f32, tag="v1")
            v1v = v1[:].rearrange("p (t o) -> p t o", o=1)
            nc.vector.tensor_tensor(v1v, eqs3[:, :, 0:1], eqs3[:, :, 1:2], OP.mult)

            valid = colpool.tile([128, NT // 4], bf16, tag="valid")
            nc.vector.tensor_tensor(valid[:], v1[:], maska[:], OP.mult)

            lab2 = colpool.tile([128, NT // 4], f32, tag="lab2")
            lab2v = lab2[:].rearrange("p (t o) -> p t o", o=1)
            nc.vector.scalar_tensor_tensor(
                lab2v, d3[:, :, 1:2], 14.0, d3[:, :, 0:1], OP.mult, OP.add)

            lab3 = colpool.tile([128, NT // 4], f32, tag="lab3")
            nc.vector.tensor_tensor(lab3[:], lab2[:], offc[:, 0:NT // 4], OP.add)

            tt = colpool.tile([128, NT // 4], f32, tag="tt")
            nc.vector.scalar_tensor_tensor(
                tt[:], lab3[:], 1.0, valid[:], OP.add, OP.mult)

            segi = colpool.tile([128, NT // 4], i16, tag="segi")
            nc.vector.tensor_scalar(segi[:], tt[:], -1.0, None, OP.add)

            # ---- one-hot via local_scatter, one per SAMPLE PAIR ----
            s4pairs = []
            for pr in range(SPC // 2):
                s4p = spool.tile([128, 2 * TPD * P], bf16, tag="s4t")
                nc.gpsimd.local_scatter(
                    s4p[:], valid[:, pr * 8:(pr + 1) * 8],
                    segi[:, pr * 8:(pr + 1) * 8],
                    channels=128, num_elems=2 * TPD * P, num_idxs=2 * TPD)
                s4pairs.append(s4p)

            lncatA = accpool.tile([128, 4 * SPC], f32, tag="lncatA")

            for k in range(SPC):
                s4full = s4pairs[k // 2]
                s4base = (k % 2) * TPD * P

                # ---- per-sample load (sample 0 preloaded) ----
                if k == 0:
                    es = es0
                else:
                    es = epool.tile([128, SW], bf16, tag="ett")
                    nc.sync.dma_start(es[:], es_in[:, k * SW:(k + 1) * SW])
                et = es[:, 0:3072]
                ipt = es[:, 3072:SW]

                # ---- G = E @ ipe^T  [tok(4/part), 196], j-chunked ----
                g1 = gpool.tile([128, TPD * GW], bf16, tag="g1t")
                g1v = g1[:].rearrange("p (j w) -> p j w", w=GW)
                nc.gpsimd.memset(g1v[:, :, P:P + 1], 1.0)
                for jj in range(2):
                    g_ps = ps_g.tile([128, 2 * P], f32, tag="g")
                    for j2 in range(2):
                        j = 2 * jj + j2
                        for ci in range(6):
                            nc.tensor.matmul(
                                g_ps[:, j2 * P:(j2 + 1) * P],
                                es[:, (ci * TPD + j) * 128:(ci * TPD + j + 1) * 128],
                                es[:, 3072 + ci * P:3072 + (ci + 1) * P],
                                start=(ci == 0), stop=(ci == 5))
                    gsv = g_ps[:].rearrange("p (a b) -> p a b", b=P)
                    dst = g1v[:, 2 * jj:2 * jj + 2, 0:P]
                    if jj == 0:
                        nc.scalar.copy(dst, gsv)
                    else:
                        nc.vector.tensor_copy(dst, gsv)

                # ---- LT_x = S^T @ [G | 1]: logits2 + counts column ----
                ltx_t = ps_lt.tile([128, 500], f32, tag="lt")
                lt1_ps = ltx_t[0:128, 0:PX]
                lt2_ps = ltx_t[0:NQ2, 250:250 + PX]
                nc.vector.memset(ltx_t[64:128, 446:447], 1.0)
                for j in range(TPD):
                    rhs = g1[:, j * GW:j * GW + PX]
                    nc.tensor.matmul(
                        lt1_ps, s4full[:, s4base + j * P:s4base + j * P + 128],
                        rhs, start=(j == 0), stop=(j == TPD - 1))
                for j in range(TPD):
                    rhs = g1[:, j * GW:j * GW + PX]
                    nc.tensor.matmul(
                        lt2_ps, s4full[:, s4base + j * P + 128:s4base + (j + 1) * P],
                        rhs, start=(j == 0), stop=(j == TPD - 1))

                # ---- inv counts: both fused count columns in one strided read
                cmax = colpool.tile([128, 2], f32, tag="cmax")
                cm3 = ltx_t[:].rearrange("p (a b) -> p a b", b=250)
                nc.vector.tensor_scalar(cmax[:], cm3[:, :, P:P + 1], 1.0, None, OP.max)
                invc = colpool.tile([128, 2], f32, tag="invc")
                nc.vector.reciprocal(invc[:], cmax[:])
                inv1 = invc[:, 0:1]
                inv2 = invc[0:NQ2, 1:2]

                # ---- scale by inv counts while copying to SBUF (bf16) ----
                ltx = ltpool.tile([128, 2 * P], bf16, tag="ltb")
                nc.gpsimd.memset(ltx[64:128, P:2 * P], 0.0)
                lt1 = ltx[0:128, 0:P]
                lt2 = ltx[0:NQ2, P:2 * P]
                nc.vector.tensor_scalar(lt1, lt1_ps[:, 0:P], inv1, None, OP.mult)
                nc.vector.tensor_scalar(lt2, lt2_ps[:, 0:P], inv2, None, OP.mult)

                # ---- diagonals of both blocks in one accumulate op ----
                djx = scrpool.tile([128, 2 * P], bf16, tag="scrw")
                nc.vector.scalar_tensor_tensor(
                    djx[:], ltx[:], 1.0, imaskc[:, 0:2 * P],
                    OP.mult, OP.mult, accum_out=dcatA[:, k:k + 1])

                # ---- colCE orientation: one merged -max, exp per block ----
                ltx3 = ltx[:].rearrange("p (a b) -> p a b", b=P)
                nc.vector.tensor_reduce(mcatA[:, 4 * k:4 * k + 2], ltx3,
                                        axis=AX.X, op=OP.max, negate=True)
                ex1 = scrpool.tile([128, P], bf16, tag="scr")
                nc.scalar.activation(ex1[:], lt1, AF.Exp,
                                     bias=mcatA[:, 4 * k:4 * k + 1], scale=1.0,
                                     accum_out=scatA[:, 4 * k:4 * k + 1])
                ex2 = scrpool.tile([NQ2, P], bf16, tag="scr2")
                nc.scalar.activation(ex2[:], lt2, AF.Exp,
                                     bias=mcatA[0:NQ2, 4 * k + 1:4 * k + 2], scale=1.0,
                                     accum_out=scatA[0:NQ2, 4 * k + 1:4 * k + 2])

                # ---- transpose LT -> L (rowCE orientation), shared bank ----
                lpx = ps_tr.tile([128, 448], bf16, tag="tr")
                lp1 = lpx[0:128, 0:P]
                lp2 = lpx[0:NQ2, 224:224 + P]
                nc.tensor.transpose(lpx[0:128, 0:128], ltx[0:128, 0:128], ident128)
                nc.tensor.transpose(lpx[0:128, 128:P], ltx[0:NQ2, P:P + 128], ident68)
                nc.tensor.transpose(lpx[0:NQ2, 224:224 + 128], ltx[0:128, 128:P], ident128)
                nc.tensor.transpose(lpx[0:NQ2, 224 + 128:224 + P],
                                    ltx[0:NQ2, P + 128:2 * P], ident68)

                nc.vector.tensor_reduce(mcatA[:, 4 * k + 2:4 * k + 3], lp1,
                                        axis=AX.X, op=OP.max, negate=True)
                ex3 = scrpool.tile([128, P], bf16, tag="scr")
                nc.scalar.activation(ex3[:], lp1, AF.Exp,
                                     bias=mcatA[:, 4 * k + 2:4 * k + 3], scale=1.0,
                                     accum_out=scatA[:, 4 * k + 2:4 * k + 3])
                nc.vector.tensor_reduce(mcatA[0:NQ2, 4 * k + 3:4 * k + 4], lp2,
                                        axis=AX.X, op=OP.max, negate=True)
                ex4 = scrpool.tile([NQ2, P], bf16, tag="scr2")
                nc.scalar.activation(ex4[:], lp2, AF.Exp,
                                     bias=mcatA[0:NQ2, 4 * k + 3:4 * k + 4], scale=1.0,
                                     accum_out=scatA[0:NQ2, 4 * k + 3:4 * k + 4])

            # ---- final reduction ----
            nc.scalar.activation(lncatA[:], scatA[:], AF.Ln)

            r1 = colpool.tile([128, 1], f32, tag="r1")
            nc.vector.tensor_reduce(r1[:], lncatA[:], axis=AX.X, op=OP.add)
            rm1 = colpool.tile([128, 1], f32, tag="rm1")
            nc.vector.tensor_reduce(rm1[:], mcatA[:], axis=AX.X, op=OP.add)
            rd1 = colpool.tile([128, 1], f32, tag="rd1")
            nc.vector.tensor_reduce(rd1[:], dcatA[:], axis=AX.X, op=OP.add)
            t1 = colpool.tile([128, 1], f32, tag="t1")
            # lse = ln(s) - negmax; rm1 holds sum of negmax columns
            nc.vector.scalar_tensor_tensor(t1[:], rm1[:], -1.0, r1[:], OP.mult, OP.add)
            tot1 = colpool.tile([128, 1], f32, tag="tot1")
            nc.vector.scalar_tensor_tensor(tot1[:], rd1[:], -2.0, t1[:], OP.mult, OP.add)

            fin_ps = ps_tr.tile([1, 1], f32, tag="tr")
            nc.tensor.matmul(fin_ps[:], tot1[:], ones_f32, start=True, stop=True)
            res = colpool.tile([1, 1], f32, tag="res")
            nc.vector.tensor_copy(res[:], fin_ps[:])
            nc.sync.dma_start(out_ext[:], res[:])

    nc.compile()
    _NC_CACHE = nc
    return nc


def _make_consts():
    import ml_dtypes
    bf = ml_dtypes.bfloat16
    imask = np.zeros((128, 392), dtype=bf)
    for p_ in range(128):
        imask[p_, p_] = 1
    for q_ in range(NQ2):
        imask[q_, P + 128 + q_] = 1   # identity cols 128..195 at offset P
    offc = np.zeros((128, 33), dtype=np.float32)
    base = np.array([0, P, 2 * P, 3 * P], dtype=np.float32)
    for s in range(SPC):
        offc[:, s * 4:(s + 1) * 4] = base[None, :] + (s % 2) * (TPD * P)
    offc[:, 32] = 1.0
    return imask, offc


def _stage_core(te, ipe, bbox, am, c):
    """Build the in_map for core c from full inputs."""
    import ml_dtypes
    bf = ml_dtypes.bfloat16
    sl = slice(c * SPC, (c + 1) * SPC)
    # et: [cp, s, ci, j, m] = E[s, 4m+j, 128*ci+cp]
    e = np.ascontiguousarray(te[sl]).reshape(SPC, 128, TPD, 6, 128)
    et = (e.transpose(4, 0, 3, 2, 1)          # [cp, s, ci, j, m]
          .reshape(128, SPC, 6 * TPD * 128)).astype(bf)
    ipet = (np.ascontiguousarray(ipe[sl]).transpose(0, 2, 1)   # [SPC, 768, 196]
            .reshape(SPC, 6, 128, P).transpose(2, 0, 1, 3)
            .reshape(128, SPC, 6 * P)).astype(bf)
    es = np.concatenate([et, ipet], axis=2).reshape(128, SPC * (6 * TPD * 128 + 6 * P))
    bbq = (bbox[sl].astype(np.int32).reshape(SPC, 128, TPD, 4)
           .transpose(1, 0, 2, 3).reshape(128, SPC * 16))
    mq = (am[sl].astype(np.float32).reshape(SPC, 128, TPD)
          .transpose(1, 0, 2).reshape(128, SPC * TPD))
    imask, offc = _make_consts()
    return {
        "es": np.ascontiguousarray(es),
        "bboxq": np.ascontiguousarray(bbq),
        "maskq": np.ascontiguousarray(mq),
        "imask": imask,
        "offc": offc,
    }


def _install_profile_hook():
    """Wire the NTFF profile hook (the image's antenv lacks axon_hooks)."""
    import sys
    import types
    try:
        import antenv.axon_hooks  # noqa: F401
        return
    except ImportError:
        pass
    import antenv
    mod = types.ModuleType("antenv.axon_hooks")
    holder = {}
    mod.set_axon_ntff_profile_hook = lambda h: holder.__setitem__("h", h)
    mod.get_axon_ntff_profile_hook = lambda: holder.get("h")
    sys.modules["antenv.axon_hooks"] = mod
    antenv.axon_hooks = mod
    from trn_agent_boot.trn_boot import _ntff_profile_via_ctypes
    mod.set_axon_ntff_profile_hook(
        _ntff_profile_via_ctypes("/opt/axon/libaxon_pjrt.so"))
    # upload_artifacts needs a bucket that doesn't exist here
    import concourse.bass_utils as bu
    bu.upload_artifacts = lambda tmpdir: f"local:{tmpdir}"


def _run(inputs, trace=False, trace_kwargs=None):
    from concourse.bass_utils import run_bass_kernel_spmd
    if trace:
        _install_profile_hook()
    te = np.asarray(inputs["text_embeds"], dtype=np.float32)
    ipe = np.asarray(inputs["image_patch_embedding"], dtype=np.float32)
    bbox = np.asarray(inputs["bbox"])
    am = np.asarray(inputs["attention_mask"])
    nc = _build_nc()
    in_maps = [_stage_core(te, ipe, bbox, am, c) for c in range(NCORES)]
    kw = {}
    if trace:
        kw = dict(trace=True, trace_kwargs=trace_kwargs or {})
    res = run_bass_kernel_spmd(nc, in_maps, core_ids=list(range(NCORES)), **kw)
    total = sum(float(res.results[i]["out"][0, 0]) for i in range(NCORES))
    loss = total / (2.0 * B * P)
    return np.asarray(loss, dtype=np.float32), res


def kernel(**inputs) -> np.ndarray:
    try:
        loss, _ = _run(inputs, trace=False)
    except Exception:
        # one retry: a previously wedged device recovers after a failed call
        loss, _ = _run(inputs, trace=False)
    return loss

